# revision 1
# baseline (speedup 1.0000x reference)
"""Trainium2 Bass kernel for nn_Dist2CycleLayer.

Computes out = relu(adjacency * Linv) @ W.T + b  with N = 8192.
(x_e is an input of the nn.Module but is discarded by its forward pass,
so it is never shipped to the device.)

Sharding: row-partition the [N, N] matrices across 8 NeuronCores
(1024 rows per core). Each core computes its 1024 output rows fully
(the reduction over the 8192 columns is row-local), outputs are
concatenated on the host.

Per-core device program (row tile = 128 partitions, column chunk = 4096):
  DMA  a = adj[rt, ch], l = linv[rt, ch]            (2 MiB each, HWDGE)
  DVE  t = a * l                                    (tensor_tensor mult)
  DVE  s = max(t, 0) * Wb ; acc[:, ch] = sum_j s    (scalar_tensor_tensor,
                                                     fused relu + weight
                                                     mult + row reduction)
  then per row tile: out = reduce_add(acc) + b, DMA out.

W is broadcast once to all 128 partitions ([128, 8192] resident in SBUF).
"""

import numpy as np

N = 8192
N_CORES = 8
ROWS = N // N_CORES  # 1024 rows per core
P = 128  # partitions
CHUNK = 4096
N_CHUNKS = N // CHUNK
N_RTILES = ROWS // P

_CACHE = {}


def _build():
    import concourse.bacc as bacc
    import concourse.mybir as mybir
    from concourse import tile

    f32 = mybir.dt.float32
    Alu = mybir.AluOpType

    nc = bacc.Bacc(
        "TRN2",
        target_bir_lowering=False,
        debug=False,
        num_devices=N_CORES,
    )

    adj = nc.dram_tensor("adj", [ROWS, N], f32, kind="ExternalInput").ap()
    linv = nc.dram_tensor("linv", [ROWS, N], f32, kind="ExternalInput").ap()
    w = nc.dram_tensor("w", [1, N], f32, kind="ExternalInput").ap()
    b = nc.dram_tensor("b", [1, 1], f32, kind="ExternalInput").ap()
    out = nc.dram_tensor("out", [ROWS, 1], f32, kind="ExternalOutput").ap()

    with tile.TileContext(nc) as tc:
        with (
            tc.tile_pool(name="consts", bufs=1) as consts,
            tc.tile_pool(name="io", bufs=2) as io,
            tc.tile_pool(name="prod", bufs=2) as prod,
            tc.tile_pool(name="sink", bufs=1) as sink,
            tc.tile_pool(name="small", bufs=2) as small,
        ):
            # W broadcast to all partitions, resident for the whole kernel.
            wb = consts.tile([P, N], f32)
            nc.sync.dma_start(out=wb[:], in_=w.broadcast_to([P, N]))
            # b broadcast to all partitions.
            b_bc = consts.tile([P, 1], f32)
            nc.sync.dma_start(out=b_bc[:], in_=b.broadcast_to([P, 1]))

            for rt in range(N_RTILES):
                r0 = rt * P
                acc = small.tile([P, N_CHUNKS], f32, tag="acc")
                for ch in range(N_CHUNKS):
                    c0 = ch * CHUNK
                    a_t = io.tile([P, CHUNK], f32, tag="a")
                    l_t = io.tile([P, CHUNK], f32, tag="l")
                    nc.sync.dma_start(
                        out=a_t[:], in_=adj[r0 : r0 + P, c0 : c0 + CHUNK]
                    )
                    nc.sync.dma_start(
                        out=l_t[:], in_=linv[r0 : r0 + P, c0 : c0 + CHUNK]
                    )
                    t = prod.tile([P, CHUNK], f32, tag="t")
                    nc.vector.tensor_mul(out=t[:], in0=a_t[:], in1=l_t[:])
                    s = sink.tile([P, CHUNK], f32, tag="s")
                    nc.vector.scalar_tensor_tensor(
                        out=s[:],
                        in0=t[:],
                        scalar=0.0,
                        in1=wb[:, c0 : c0 + CHUNK],
                        op0=Alu.max,
                        op1=Alu.mult,
                        accum_out=acc[:, ch : ch + 1],
                    )
                res = small.tile([P, 1], f32, tag="res")
                nc.vector.tensor_reduce(
                    out=res[:], in_=acc[:], axis=mybir.AxisListType.X, op=Alu.add
                )
                res2 = small.tile([P, 1], f32, tag="res2")
                nc.vector.tensor_add(out=res2[:], in0=res[:], in1=b_bc[:])
                nc.sync.dma_start(out=out[r0 : r0 + P, :], in_=res2[:])

    nc.compile()
    return nc


def get_nc():
    if "nc" not in _CACHE:
        _CACHE["nc"] = _build()
    return _CACHE["nc"]


def make_in_maps(adjacency, Linv, W, b):
    adjacency = np.ascontiguousarray(adjacency, dtype=np.float32)
    Linv = np.ascontiguousarray(Linv, dtype=np.float32)
    W = np.ascontiguousarray(W, dtype=np.float32).reshape(1, N)
    b = np.ascontiguousarray(b, dtype=np.float32).reshape(1, 1)
    in_maps = []
    for c in range(N_CORES):
        r0, r1 = c * ROWS, (c + 1) * ROWS
        in_maps.append(
            {
                "adj": adjacency[r0:r1],
                "linv": Linv[r0:r1],
                "w": W,
                "b": b,
            }
        )
    return in_maps


def kernel(x_e=None, Linv=None, adjacency=None, W=None, b=None, **_unused):
    from concourse.bass_utils import run_bass_kernel_spmd

    nc = get_nc()
    in_maps = make_in_maps(adjacency, Linv, W, b)
    res = run_bass_kernel_spmd(nc, in_maps, core_ids=list(range(N_CORES)))
    out = np.concatenate([r["out"] for r in res.results], axis=0)
    return out.astype(np.float32)
